# revision 21
# baseline (speedup 1.0000x reference)
"""DeformGNN forward pass on 8 TRN2 NeuronCores.

Sharding: data-parallel over batch B=2 (cores 0-3 = sample 0, cores 4-7 =
sample 1); within each sample, the N=2048 graph nodes are row-split 4 ways
(512 rows/core). Each gconv layer computes its row block locally (the
adjacency matmul uses a resident bf16 adj^T column slice in SBUF) and the
per-layer node states are exchanged with a 4-rank AllGather of the
natural-layout bf16 state. Per layer, the state transposes, fc1 and the
own-block part of the adjacency matmul are scheduled during the collective
flight; only the peer-block matmuls wait on the gathered data.

All arithmetic runs on device; the host only slices / transposes /
zero-pads for sharding and concatenates shards on the way out.
"""
import numpy as np

from concourse import bass, mybir, bacc, tile
from concourse.bass_utils import run_bass_kernel_spmd
from concourse.bass_interp import get_hw_module
from concourse.masks import make_identity

F32 = mybir.dt.float32
BF16 = mybir.dt.bfloat16
I32 = mybir.dt.int32
AF = mybir.ActivationFunctionType
OP = mybir.AluOpType

B, N, H, W = 2, 2048, 256, 256
C, S, MID = 128, 256, 6
RPC = N // 4          # rows per core
NKT = N // 128        # 16 k-tiles over nodes
NL = 14               # gconv layers
GROUPS = [[0, 1, 2, 3], [4, 5, 6, 7]]

_BUILD_CACHE = {}


def _build():
    if "nc" in _BUILD_CACHE:
        return _BUILD_CACHE["nc"]
    nc = bacc.Bacc("TRN2", target_bir_lowering=False, debug=False, num_devices=8)

    feat = nc.dram_tensor("feat", [H * W, C], F32, kind="ExternalInput").ap()
    adjT_d = nc.dram_tensor("adjT", [N, RPC], F32, kind="ExternalInput").ap()
    bp_d = nc.dram_tensor("bp", [RPC, 2], F32, kind="ExternalInput").ap()
    bpT_d = nc.dram_tensor("bpT", [2, RPC], F32, kind="ExternalInput").ap()
    maskT_d = nc.dram_tensor("maskT", [2, RPC], F32, kind="ExternalInput").ap()
    wst_d = nc.dram_tensor("wst", [NL, 2, S, S], F32, kind="ExternalInput").ap()
    # bias rows: [2, NL*S] fp32, row j holds layer-l bias at cols l*S..(l+1)*S
    bias_d = nc.dram_tensor("bias", [2, NL * S], F32, kind="ExternalInput").ap()
    wfc_d = nc.dram_tensor("wfc", [128, 4], F32, kind="ExternalInput").ap()
    bfc_d = nc.dram_tensor("bfc", [2, 1], F32, kind="ExternalInput").ap()

    out_pred = nc.dram_tensor("out_pred", [2, RPC], F32, kind="ExternalOutput").ap()
    out_gcn = nc.dram_tensor("out_gcn", [2, RPC], F32, kind="ExternalOutput").ap()
    out_lap = nc.dram_tensor("out_lap", [1, 1], F32, kind="ExternalOutput").ap()

    # SPMD note: all 8 cores run one program, so the "own block" adjacency
    # matmul cannot use a per-core row offset into a shared global adjT.
    # Instead the host passes, per core: adjT_own = adj^T rows of the own
    # 512-node chunk (dense), and adjT = the full-global-order adj^T slice
    # with the own chunk rows ZEROED. The own-block matmul (which overlaps
    # the collective) uses adjT_own with the local state tile; the gathered
    # blocks run over all four chunks of adjT, where the own chunk
    # contributes zero. Static program, per-core data.
    adjT_own_d = nc.dram_tensor("adjT_own", [RPC, RPC], F32,
                                kind="ExternalInput").ap()

    with tile.TileContext(nc) as tc:
        with tc.tile_pool(name="persist", bufs=1) as pp, \
             tc.tile_pool(name="adjp", bufs=NKT + 4) as adjp, \
             tc.tile_pool(name="wp", bufs=2 * NL * 2) as wp, \
             tc.tile_pool(name="xnp", bufs=4) as xnp, \
             tc.tile_pool(name="xfp", bufs=8) as xfp, \
             tc.tile_pool(name="xtp", bufs=4) as xtp, \
             tc.tile_pool(name="msp", bufs=4) as msp, \
             tc.tile_pool(name="gthp", bufs=8) as gthp, \
             tc.tile_pool(name="smp", bufs=32) as smp, \
             tc.tile_pool(name="psM", bufs=2, space="PSUM") as psM, \
             tc.tile_pool(name="psG", bufs=4, space="PSUM") as psG, \
             tc.tile_pool(name="psT", bufs=2, space="PSUM") as psT, \
             tc.tile_pool(name="dr", bufs=4, space="DRAM") as dr:

            # ---------- constants ----------
            ident = pp.tile([128, 128], F32, name="ident")
            make_identity(nc, ident[:])
            ident_bf = pp.tile([128, 128], BF16, name="ident_bf")
            make_identity(nc, ident_bf[:])
            eps = pp.tile([1, 1], F32, name="eps")
            nc.vector.memset(eps[:], 1e-10)
            z128 = pp.tile([128, 1], F32, name="z128")
            nc.vector.memset(z128[:], 0.0)
            ones1 = pp.tile([1, 128], BF16, name="ones1")
            nc.vector.memset(ones1[:], 1.0)

            # ---------- bilinear interpolation -> x0 natural (own rows) ----
            # natural state layout: [128, 4*S]; row-tile t occupies columns
            # [t*S, (t+1)*S) = 256 state dims for the 128 points of tile t.
            x0n = xnp.tile([128, 4 * S], BF16, tag="xn", name="x0n")
            nc.vector.memset(x0n[:], 0.0)

            bp_sb = pp.tile([128, 8], F32, name="bp_sb")
            for t in range(4):
                nc.sync.dma_start(out=bp_sb[:, 2 * t:2 * t + 2],
                                  in_=bp_d[128 * t:128 * (t + 1), :])

            for t in range(4):
                bpx = bp_sb[:, 2 * t:2 * t + 1]
                bpy = bp_sb[:, 2 * t + 1:2 * t + 2]
                Xs = smp.tile([128, 1], F32, tag="sm")
                Ys = smp.tile([128, 1], F32, tag="sm")
                nc.vector.tensor_scalar_mul(out=Xs[:], in0=bpx, scalar1=float(W))
                nc.vector.tensor_scalar_mul(out=Ys[:], in0=bpy, scalar1=float(H))

                def floor_(v):
                    vi = smp.tile([128, 1], I32, tag="smi")
                    nc.vector.tensor_copy(out=vi[:], in_=v[:])
                    vf = smp.tile([128, 1], F32, tag="sm")
                    nc.vector.tensor_copy(out=vf[:], in_=vi[:])
                    corr = smp.tile([128, 1], F32, tag="sm")
                    nc.vector.tensor_tensor(out=corr[:], in0=vf[:], in1=v[:],
                                            op=OP.is_gt)
                    v0 = smp.tile([128, 1], F32, tag="sm")
                    nc.vector.tensor_tensor(out=v0[:], in0=vf[:], in1=corr[:],
                                            op=OP.subtract)
                    return v0

                X0f = floor_(Xs)
                Y0f = floor_(Ys)
                fx = smp.tile([128, 1], F32, tag="sm")
                fy = smp.tile([128, 1], F32, tag="sm")
                nc.vector.tensor_tensor(out=fx[:], in0=Xs[:], in1=X0f[:],
                                        op=OP.subtract)
                nc.vector.tensor_tensor(out=fy[:], in0=Ys[:], in1=Y0f[:],
                                        op=OP.subtract)
                gx = smp.tile([128, 1], F32, tag="sm")
                gy = smp.tile([128, 1], F32, tag="sm")
                nc.vector.tensor_scalar(out=gx[:], in0=fx[:], scalar1=-1.0,
                                        scalar2=1.0, op0=OP.mult, op1=OP.add)
                nc.vector.tensor_scalar(out=gy[:], in0=fy[:], scalar1=-1.0,
                                        scalar2=1.0, op0=OP.mult, op1=OP.add)
                ws = {}
                for nmx, wx in (("0", gx), ("1", fx)):
                    for nmy, wy in (("0", gy), ("1", fy)):
                        wt = smp.tile([128, 1], F32, tag="sm")
                        nc.vector.tensor_tensor(out=wt[:], in0=wx[:], in1=wy[:],
                                                op=OP.mult)
                        ws[nmx + nmy] = wt

                def clip_(v, hi):
                    vc = smp.tile([128, 1], F32, tag="sm")
                    nc.vector.tensor_scalar(out=vc[:], in0=v[:], scalar1=0.0,
                                            scalar2=float(hi), op0=OP.max,
                                            op1=OP.min)
                    return vc
                X1f = smp.tile([128, 1], F32, tag="sm")
                Y1f = smp.tile([128, 1], F32, tag="sm")
                nc.vector.tensor_scalar_add(out=X1f[:], in0=X0f[:], scalar1=1.0)
                nc.vector.tensor_scalar_add(out=Y1f[:], in0=Y0f[:], scalar1=1.0)
                Xc = {"0": clip_(X0f, W - 1), "1": clip_(X1f, W - 1)}
                Yc = {"0": clip_(Y0f, H - 1), "1": clip_(Y1f, H - 1)}
                acc = gthp.tile([128, C], F32, tag="acc")
                first = True
                for nmx in ("0", "1"):
                    for nmy in ("0", "1"):
                        idxf = smp.tile([128, 1], F32, tag="sm")
                        nc.vector.tensor_scalar(out=idxf[:], in0=Yc[nmy][:],
                                                scalar1=float(W), scalar2=None,
                                                op0=OP.mult)
                        nc.vector.tensor_tensor(out=idxf[:], in0=idxf[:],
                                                in1=Xc[nmx][:], op=OP.add)
                        idx = smp.tile([128, 1], I32, tag="smi")
                        nc.vector.tensor_copy(out=idx[:], in_=idxf[:])
                        g = gthp.tile([128, C], F32, tag="gth")
                        nc.gpsimd.indirect_dma_start(
                            out=g[:], out_offset=None, in_=feat,
                            in_offset=bass.IndirectOffsetOnAxis(ap=idx[:, :1],
                                                                axis=0))
                        if first:
                            nc.vector.tensor_scalar_mul(
                                out=acc[:], in0=g[:], scalar1=ws[nmx + nmy][:, :1])
                            first = False
                        else:
                            gw = gthp.tile([128, C], F32, tag="gw")
                            nc.vector.tensor_scalar_mul(
                                out=gw[:], in0=g[:], scalar1=ws[nmx + nmy][:, :1])
                            nc.vector.tensor_tensor(out=acc[:], in0=acc[:],
                                                    in1=gw[:], op=OP.add)
                nc.vector.tensor_copy(out=x0n[:, t * S:t * S + C], in_=acc[:])
                nc.vector.tensor_copy(out=x0n[:, t * S + C:t * S + C + 2],
                                      in_=bp_sb[:, 2 * t:2 * t + 2])

            # ---------- small parameter loads ----------
            bias_j0 = pp.tile([1, NL * S], F32, name="bias_j0")
            bias_j1 = pp.tile([1, NL * S], F32, name="bias_j1")
            nc.sync.dma_start(out=bias_j0[:], in_=bias_d[0:1, :])
            nc.sync.dma_start(out=bias_j1[:], in_=bias_d[1:2, :])
            bcomb = pp.tile([1, NL * S], F32, name="bcomb")
            nc.vector.tensor_tensor(out=bcomb[:], in0=bias_j0[:], in1=bias_j1[:],
                                    op=OP.add)
            bcomb_bf = pp.tile([1, NL * S], BF16, name="bcomb_bf")
            nc.vector.tensor_copy(out=bcomb_bf[:], in_=bcomb[:])
            wfc_sb = pp.tile([128, 4], BF16, name="wfc_sb")
            nc.gpsimd.dma_start(out=wfc_sb[:], in_=wfc_d)
            bfc_sb = pp.tile([2, 1], F32, name="bfc_sb")
            nc.sync.dma_start(out=bfc_sb[:], in_=bfc_d)
            maskT_sb = pp.tile([2, RPC], F32, name="maskT_sb")
            nc.sync.dma_start(out=maskT_sb[:], in_=maskT_d)
            bpT_sb = pp.tile([2, RPC], F32, name="bpT_sb")
            nc.sync.dma_start(out=bpT_sb[:], in_=bpT_d)

            def exchange(xn_tile):
                """AllGather the natural-layout own block; return (agout,
                x_full chunk tiles read back in global chunk order)."""
                agin = dr.tile([RPC, S], BF16, tag="agin")
                nc.sync.dma_start(
                    out=agin[:].rearrange("(t p) d -> p t d", p=128),
                    in_=xn_tile[:].rearrange("p (t d) -> p t d", t=4))
                agout = dr.tile([N, S], BF16, tag="agout")
                nc.gpsimd.collective_compute(
                    "AllGather", OP.bypass, replica_groups=GROUPS,
                    ins=[agin[:].opt()], outs=[agout[:].opt()])
                xfull = []
                for r in range(4):
                    xr = xfp.tile([128, 4 * S], BF16, tag="xf",
                                  name=f"xr_{r}")
                    ring = nc.scalar if r % 2 == 0 else nc.sync
                    ring.dma_start(
                        out=xr[:].rearrange("p (t d) -> p t d", t=4),
                        in_=agout[RPC * r:RPC * (r + 1), :]
                        .rearrange("(t p) d -> p t d", p=128))
                    xfull.append(xr)
                return xfull

            xfull = exchange(x0n)

            # ---------- persistent adjacency + weights (overlap AG0) ------
            # adjT_sb[k]: global k-tile rows of adj^T own-column slice, with
            # the own chunk rows zeroed by the host. Loaded fp32 on the sync
            # ring (concurrent with the gpsimd gathers) and cast on DVE.
            adjT_sb = [adjp.tile([128, RPC], BF16, tag="adjT", name=f"adjT{k}")
                       for k in range(NKT)]
            adjT_own = [adjp.tile([128, RPC], BF16, tag="adjT", name=f"adjTo{k}")
                        for k in range(4)]
            adj_loads = [(adjT_own[k], adjT_own_d, k) for k in range(4)] + \
                        [(adjT_sb[k], adjT_d, k) for k in range(NKT)]
            for li, (dst, src, k) in enumerate(adj_loads):
                stg = gthp.tile([128, RPC], F32, tag="adjstage",
                                name=f"astg{li}")
                nc.sync.dma_start(out=stg[:],
                                  in_=src[128 * k:128 * (k + 1), :])
                nc.vector.tensor_copy(out=dst[:], in_=stg[:])

            w_sb = {}
            for l in range(NL):
                for j in range(2):
                    for kt in range(2):
                        t_ = wp.tile([128, S], BF16, tag="w", name=f"w{l}_{j}_{kt}")
                        nc.gpsimd.dma_start(
                            out=t_[:],
                            in_=wst_d[l:l + 1, j:j + 1,
                                      128 * kt:128 * (kt + 1), :].opt())
                        w_sb[(l, j, kt)] = t_

            # ---------- gconv layers ----------
            xn_cur = x0n
            blockin = None
            for l in range(NL):
                if l == 0 or l == NL - 1:
                    kind = "plain"
                elif l % 2 == 1:
                    kind = "relu"
                    blockin = xn_cur
                else:
                    kind = "resid"

                # --- state transposes (local; overlap previous AG) ---
                xT = [xtp.tile([128, RPC], BF16, tag="xt", name=f"xT{l}_{i}")
                      for i in range(2)]
                for t in range(4):
                    for mt in range(2):
                        ptx = psT.tile([128, 128], BF16, tag="psT", space="PSUM")
                        nc.tensor.transpose(
                            out=ptx[:],
                            in_=xn_cur[:, t * S + mt * 128:t * S + (mt + 1) * 128],
                            identity=ident_bf[:])
                        nc.vector.tensor_copy(out=xT[mt][:, 128 * t:128 * (t + 1)],
                                              in_=ptx[:])

                # --- fc1 into psum (local; overlap previous AG) ---
                pg = [psG.tile([128, S], F32, tag="psG", space="PSUM",
                               name=f"pg{l}_{i}") for i in range(4)]
                for Mt in range(4):
                    for kt2 in range(2):
                        nc.tensor.matmul(
                            out=pg[Mt][:],
                            lhsT=xT[kt2][:, 128 * Mt:128 * (Mt + 1)],
                            rhs=w_sb[(l, 0, kt2)][:],
                            start=(kt2 == 0), stop=False)
                    # bias row (K=1 broadcast matmul)
                    nc.tensor.matmul(
                        out=pg[Mt][:], lhsT=ones1[:],
                        rhs=bcomb_bf[0:1, S * l:S * (l + 1)],
                        start=False, stop=False)

                # --- adjacency matmul m^T [2 x [128, 512]] ---
                pm = [psM.tile([128, RPC], F32, tag="psM", space="PSUM",
                               name=f"pm{l}_{i}") for i in range(2)]
                # own block first (local state, overlaps previous AG)
                for mt in range(2):
                    for kt in range(4):
                        nc.tensor.matmul(
                            out=pm[mt][:],
                            lhsT=xn_cur[:, kt * S + mt * 128:kt * S + (mt + 1) * 128],
                            rhs=adjT_own[kt][:],
                            start=(kt == 0), stop=False)
                # warm-keeper: dummy matmuls on resident tiles spanning the
                # collective wait, so the PE's HAM clock gate stays at full
                # rate for the peer-block matmuls that follow. Results land
                # in a scratch PSUM bank and are never read.
                pdum = psT.tile([128, RPC], F32, tag="psT", space="PSUM",
                                name=f"pdum{l}")
                for w_i in range(20):
                    nc.tensor.matmul(out=pdum[:], lhsT=ident_bf[:],
                                     rhs=adjT_own[w_i % 4][:],
                                     start=True, stop=True)

                # gathered blocks (own chunk rows of adjT_sb are zeroed)
                for r in range(4):
                    for kt in range(4):
                        gk = r * 4 + kt
                        for mt in range(2):
                            nc.tensor.matmul(
                                out=pm[mt][:],
                                lhsT=xfull[r][:, kt * S + mt * 128:
                                              kt * S + (mt + 1) * 128],
                                rhs=adjT_sb[gk][:],
                                start=False,
                                stop=(r == 3 and kt == 3))
                m_sb = []
                for mt in range(2):
                    ms = msp.tile([128, RPC], BF16, tag="ms")
                    nc.vector.tensor_copy(out=ms[:], in_=pm[mt][:])
                    m_sb.append(ms)

                # --- fc2 + epilogue ---
                xn_next = xnp.tile([128, 4 * S], BF16, tag="xn")
                for Mt in range(4):
                    for kt2 in range(2):
                        nc.tensor.matmul(
                            out=pg[Mt][:],
                            lhsT=m_sb[kt2][:, 128 * Mt:128 * (Mt + 1)],
                            rhs=w_sb[(l, 1, kt2)][:],
                            start=False, stop=(kt2 == 1))
                    dst = xn_next[:, Mt * S:(Mt + 1) * S]
                    if kind == "relu":
                        nc.scalar.activation(out=dst, in_=pg[Mt][:], func=AF.Relu,
                                             bias=z128[:])
                    elif kind == "plain":
                        nc.scalar.activation(out=dst, in_=pg[Mt][:],
                                             func=AF.Identity, bias=z128[:])
                    else:
                        t2 = msp.tile([128, S], BF16, tag="t2")
                        nc.scalar.activation(out=t2[:], in_=pg[Mt][:],
                                             func=AF.Identity, bias=z128[:])
                        t3 = msp.tile([128, S], BF16, tag="t2")
                        nc.vector.tensor_tensor(
                            out=t3[:], in0=t2[:],
                            in1=blockin[:, Mt * S:(Mt + 1) * S], op=OP.add)
                        nc.vector.tensor_scalar(out=dst, in0=t3[:], scalar1=0.0,
                                                scalar2=None, op0=OP.max)

                if l < NL - 1:
                    xfull = exchange(xn_next)
                xn_cur = xn_next

            # ---------- head: gcn_pred, pred_points ----------
            x14T = [xtp.tile([128, RPC], BF16, tag="xt", name=f"x14T_{i}")
                    for i in range(2)]
            for t in range(4):
                for mt in range(2):
                    ptx = psT.tile([128, 128], BF16, tag="psT", space="PSUM")
                    nc.tensor.transpose(
                        out=ptx[:],
                        in_=xn_cur[:, t * S + mt * 128:t * S + (mt + 1) * 128],
                        identity=ident_bf[:])
                    nc.vector.tensor_copy(out=x14T[mt][:, 128 * t:128 * (t + 1)],
                                          in_=ptx[:])
            pgc = psT.tile([2, RPC], F32, tag="psT", space="PSUM")
            nc.tensor.matmul(out=pgc[:], lhsT=wfc_sb[:, 0:2], rhs=x14T[0][:],
                             start=True, stop=False)
            nc.tensor.matmul(out=pgc[:], lhsT=wfc_sb[:, 2:4], rhs=x14T[1][:],
                             start=False, stop=True)
            gcn_T = pp.tile([2, RPC], F32, name="gcn_T")
            nc.scalar.activation(out=gcn_T[:], in_=pgc[:], func=AF.Identity,
                                 bias=bfc_sb[:])
            nc.sync.dma_start(out=out_gcn, in_=gcn_T[:])
            d_T = pp.tile([2, RPC], F32, name="d_T")
            nc.vector.tensor_tensor(out=d_T[:], in0=gcn_T[:], in1=maskT_sb[:],
                                    op=OP.mult)
            pred_T = pp.tile([2, RPC], F32, name="pred_T")
            nc.vector.tensor_tensor(out=pred_T[:], in0=bpT_sb[:], in1=d_T[:],
                                    op=OP.add)
            nc.sync.dma_start(out=out_pred, in_=pred_T[:])

            # ---------- laplacian energy ----------
            d_T_bf = pp.tile([2, RPC], BF16, name="d_T_bf")
            nc.vector.tensor_copy(out=d_T_bf[:], in_=d_T[:])
            agin_d = dr.tile([RPC, 2], BF16, tag="agind")
            for t in range(4):
                ptd = psT.tile([128, 2], BF16, tag="psT", space="PSUM")
                nc.tensor.transpose(out=ptd[:, 0:2],
                                    in_=d_T_bf[:, 128 * t:128 * (t + 1)],
                                    identity=ident_bf[0:2, 0:2])
                dnn = smp.tile([128, 2], BF16, tag="dn")
                nc.vector.tensor_copy(out=dnn[:], in_=ptd[:, 0:2])
                nc.sync.dma_start(out=agin_d[128 * t:128 * (t + 1), :], in_=dnn[:])
            agout_d = dr.tile([N, 2], BF16, tag="agoutd")
            nc.gpsimd.collective_compute(
                "AllGather", OP.bypass, replica_groups=GROUPS,
                ins=[agin_d[:].opt()], outs=[agout_d[:].opt()])
            # (adj @ d)^T via global-order adjT (own rows zeroed) + own rows
            pad = psT.tile([2, RPC], F32, tag="psT", space="PSUM")
            for kt in range(NKT):
                dn = smp.tile([128, 2], BF16, tag="dn")
                nc.sync.dma_start(out=dn[:],
                                  in_=agout_d[128 * kt:128 * (kt + 1), :])
                nc.tensor.matmul(out=pad[:], lhsT=dn[:], rhs=adjT_sb[kt][:],
                                 start=(kt == 0), stop=False)
            # own-chunk contribution (host zeroed those rows in adjT_d):
            # own rows of d in global order are unknown per-core, BUT
            # adjT_own covers exactly the own chunk rows; the own-chunk d
            # values are d_nat own = transposed d_T (already in agin_d
            # order = own rows 0..511). Read them back from agin_d.
            for kt in range(4):
                dno = smp.tile([128, 2], BF16, tag="dn")
                nc.sync.dma_start(out=dno[:],
                                  in_=agin_d[128 * kt:128 * (kt + 1), :])
                nc.tensor.matmul(out=pad[:], lhsT=dno[:], rhs=adjT_own[kt][:],
                                 start=False, stop=(kt == 3))
            lapd = pp.tile([2, RPC], F32, name="lapd")
            nc.vector.tensor_tensor(out=lapd[:], in0=d_T[:], in1=pad[:],
                                    op=OP.subtract)
            sq = pp.tile([2, RPC], F32, name="sq")
            nc.vector.tensor_tensor(out=sq[:], in0=lapd[:], in1=lapd[:],
                                    op=OP.mult)
            ones2 = pp.tile([2, 1], F32, name="ones2")
            nc.vector.memset(ones2[:], 1.0)
            pe2 = psT.tile([1, RPC], F32, tag="psT", space="PSUM")
            nc.tensor.matmul(out=pe2[:], lhsT=ones2[:], rhs=sq[:],
                             start=True, stop=True)
            esb = pp.tile([1, RPC], F32, name="esb")
            part = pp.tile([1, 1], F32, name="part")
            nc.scalar.activation(out=esb[:], in_=pe2[:], func=AF.Sqrt,
                                 bias=eps[:], accum_out=part[:])
            agin_e_sb = pp.tile([1, 16], F32, name="agin_e_sb")
            nc.vector.memset(agin_e_sb[:], 0.0)
            nc.vector.tensor_scalar_mul(out=agin_e_sb[0:1, 0:1], in0=part[:],
                                        scalar1=1.0 / N)
            agin_e = dr.tile([1, 16], F32, tag="agine")
            nc.sync.dma_start(out=agin_e[:], in_=agin_e_sb[:])
            agout_e = dr.tile([4, 16], F32, tag="agoute")
            nc.gpsimd.collective_compute(
                "AllGather", OP.bypass, replica_groups=GROUPS,
                ins=[agin_e[:].opt()], outs=[agout_e[:].opt()])
            pe4 = pp.tile([4, 16], F32, name="pe4")
            nc.sync.dma_start(out=pe4[:], in_=agout_e[:])
            lap_sb = pp.tile([1, 1], F32, name="lap_sb")
            nc.gpsimd.tensor_reduce(out=lap_sb[:], in_=pe4[:, 0:1],
                                    axis=mybir.AxisListType.C, op=OP.add)
            nc.sync.dma_start(out=out_lap, in_=lap_sb[:])

    nc.compile()
    nc.m = get_hw_module(nc.m)
    _BUILD_CACHE["nc"] = nc
    return nc


def _prep_in_maps(inputs):
    features = np.asarray(inputs["features"], np.float32)
    base_point = np.asarray(inputs["base_point"], np.float32)
    adj = np.asarray(inputs["adj"], np.float32)
    mask = np.asarray(inputs["point_mask"], np.float32)
    w_first = np.asarray(inputs["w_first"], np.float32)
    b_first = np.asarray(inputs["b_first"], np.float32)
    w_mid = np.asarray(inputs["w_mid"], np.float32)
    b_mid = np.asarray(inputs["b_mid"], np.float32)
    w_last = np.asarray(inputs["w_last"], np.float32)
    b_last = np.asarray(inputs["b_last"], np.float32)
    w_fc = np.asarray(inputs["w_fc"], np.float32)
    b_fc = np.asarray(inputs["b_fc"], np.float32)

    wstack = np.zeros((NL, 2, S, S), np.float32)
    bstack = np.zeros((NL, 2, S), np.float32)
    wstack[0, :, :w_first.shape[1], :] = w_first
    bstack[0] = b_first
    for i in range(MID):
        wstack[1 + 2 * i, 0] = w_mid[i, 0]; wstack[1 + 2 * i, 1] = w_mid[i, 1]
        wstack[2 + 2 * i, 0] = w_mid[i, 2]; wstack[2 + 2 * i, 1] = w_mid[i, 3]
        bstack[1 + 2 * i, 0] = b_mid[i, 0]; bstack[1 + 2 * i, 1] = b_mid[i, 1]
        bstack[2 + 2 * i, 0] = b_mid[i, 2]; bstack[2 + 2 * i, 1] = b_mid[i, 3]
    wstack[NL - 1] = w_last
    bstack[NL - 1] = b_last
    bias_host = np.zeros((2, NL * S), np.float32)
    for l in range(NL):
        for j in range(2):
            bias_host[j, l * S:(l + 1) * S] = bstack[l, j]
    wfc_host = np.concatenate([w_fc[0:128, :], w_fc[128:256, :]], axis=1)
    bfc_host = b_fc[:, None]

    feats = [np.ascontiguousarray(features[b].reshape(C, H * W).T)
             for b in range(B)]
    in_maps = []
    for c in range(8):
        b, r = c // 4, c % 4
        rows = slice(r * RPC, (r + 1) * RPC)
        adjT_full = np.ascontiguousarray(adj[b, rows, :].T)   # [2048, 512]
        adjT_own = np.ascontiguousarray(adjT_full[rows, :])   # [512, 512]
        adjT_masked = adjT_full.copy()
        adjT_masked[rows, :] = 0.0
        in_maps.append({
            "feat": feats[b],
            "adjT": adjT_masked,
            "adjT_own": adjT_own,
            "bp": np.ascontiguousarray(base_point[b, rows, :]),
            "bpT": np.ascontiguousarray(base_point[b, rows, :].T),
            "maskT": np.ascontiguousarray(
                np.broadcast_to(mask[b, 0, rows, 0][None, :], (2, RPC))),
            "wst": wstack,
            "bias": bias_host,
            "wfc": wfc_host,
            "bfc": bfc_host,
        })
    return in_maps


def _install_profile_hook():
    """Register the axon NTFF profile hook (missing antenv.axon_hooks in
    this image) so run_bass_kernel_spmd(trace=True) can report exec_time_ns."""
    import sys
    import types
    import contextlib
    import ctypes

    if "antenv.axon_hooks" in sys.modules:
        return
    so_path = "/opt/axon/libaxon_pjrt.so"
    lib = ctypes.CDLL(so_path)
    if not hasattr(lib, "axon_start_nrt_profile"):
        return
    lib.axon_start_nrt_profile.argtypes = [ctypes.POINTER(ctypes.c_int64),
                                           ctypes.c_size_t]
    lib.axon_start_nrt_profile.restype = ctypes.c_int64
    lib.axon_stop_nrt_profile.argtypes = [ctypes.c_char_p]
    lib.axon_stop_nrt_profile.restype = ctypes.c_int64

    @contextlib.contextmanager
    def _hook(output_dir, device_ids):
        import jax
        jax.devices()
        if device_ids:
            ids = (ctypes.c_int64 * len(device_ids))(*device_ids)
            rc = lib.axon_start_nrt_profile(ids, len(device_ids))
        else:
            rc = lib.axon_start_nrt_profile(None, 0)
        if rc != 0:
            raise RuntimeError(f"axon_start_nrt_profile rc={rc}")
        try:
            yield
        finally:
            n = lib.axon_stop_nrt_profile(str(output_dir).encode())
            print(f"profile: {n} file(s) written to {output_dir}")

    mod = types.ModuleType("antenv.axon_hooks")
    mod.get_axon_ntff_profile_hook = lambda: _hook
    mod.set_axon_ntff_profile_hook = lambda h: None
    sys.modules["antenv.axon_hooks"] = mod

    from concourse import bass_utils as bu
    bu.upload_artifacts = lambda tmpdir: str(tmpdir)


def kernel(**inputs):
    import os
    nc = _build()
    in_maps = _prep_in_maps(inputs)
    trace = bool(os.environ.get("KERNEL_TRACE"))
    if trace:
        _install_profile_hook()
    res = run_bass_kernel_spmd(nc, in_maps, core_ids=list(range(8)),
                               trace=trace)
    if trace:
        print(f"HW exec time: {res.exec_time_ns} ns "
              f"(mean {res.mean_exec_time_ns}, max core "
              f"{res.max_exec_time_core_id})")
    pred = np.zeros((B, N, 2), np.float32)
    gcn = np.zeros((B, N, 2), np.float32)
    lap = np.zeros((B,), np.float32)
    for c in range(8):
        b, r = c // 4, c % 4
        rows = slice(r * RPC, (r + 1) * RPC)
        pred[b, rows, :] = res.results[c]["out_pred"].T
        gcn[b, rows, :] = res.results[c]["out_gcn"].T
        if r == 0:
            lap[b] = res.results[c]["out_lap"][0, 0]
    return pred, gcn, lap


# revision 25
# speedup vs baseline: 1.0509x; 1.0509x over previous
"""DeformGNN forward pass on 8 TRN2 NeuronCores.

Sharding: data-parallel over batch B=2 (cores 0-3 = sample 0, cores 4-7 =
sample 1); within each sample, the N=2048 graph nodes are row-split 4 ways
(512 rows/core). Each gconv layer computes its row block locally (the
adjacency matmul uses a resident bf16 adj^T column slice in SBUF) and the
per-layer node states are exchanged with a 4-rank AllGather of the
natural-layout bf16 state. Per layer, the state transposes, fc1 and the
own-block part of the adjacency matmul are scheduled during the collective
flight; only the peer-block matmuls wait on the gathered data.

All arithmetic runs on device; the host only slices / transposes /
zero-pads for sharding and concatenates shards on the way out.
"""
import numpy as np

from concourse import bass, mybir, bacc, tile
from concourse.bass_utils import run_bass_kernel_spmd
from concourse.bass_interp import get_hw_module
from concourse.masks import make_identity

F32 = mybir.dt.float32
BF16 = mybir.dt.bfloat16
I32 = mybir.dt.int32
AF = mybir.ActivationFunctionType
OP = mybir.AluOpType

B, N, H, W = 2, 2048, 256, 256
C, S, MID = 128, 256, 6
RPC = N // 4          # rows per core
NKT = N // 128        # 16 k-tiles over nodes
NL = 14               # gconv layers
GROUPS = [[0, 1, 2, 3], [4, 5, 6, 7]]

_BUILD_CACHE = {}


def _build():
    if "nc" in _BUILD_CACHE:
        return _BUILD_CACHE["nc"]
    nc = bacc.Bacc("TRN2", target_bir_lowering=False, debug=False, num_devices=8)

    feat = nc.dram_tensor("feat", [H * W, C], F32, kind="ExternalInput").ap()
    adjT_d = nc.dram_tensor("adjT", [N, RPC], F32, kind="ExternalInput").ap()
    bp_d = nc.dram_tensor("bp", [RPC, 2], F32, kind="ExternalInput").ap()
    bpT_d = nc.dram_tensor("bpT", [2, RPC], F32, kind="ExternalInput").ap()
    maskT_d = nc.dram_tensor("maskT", [2, RPC], F32, kind="ExternalInput").ap()
    wst_d = nc.dram_tensor("wst", [NL, 2, S, S], F32, kind="ExternalInput").ap()
    # bias rows: [2, NL*S] fp32, row j holds layer-l bias at cols l*S..(l+1)*S
    bias_d = nc.dram_tensor("bias", [2, NL * S], F32, kind="ExternalInput").ap()
    wfc_d = nc.dram_tensor("wfc", [128, 4], F32, kind="ExternalInput").ap()
    bfc_d = nc.dram_tensor("bfc", [2, 1], F32, kind="ExternalInput").ap()

    out_pred = nc.dram_tensor("out_pred", [2, RPC], F32, kind="ExternalOutput").ap()
    out_gcn = nc.dram_tensor("out_gcn", [2, RPC], F32, kind="ExternalOutput").ap()
    out_lap = nc.dram_tensor("out_lap", [1, 1], F32, kind="ExternalOutput").ap()

    # SPMD note: all 8 cores run one program, so the "own block" adjacency
    # matmul cannot use a per-core row offset into a shared global adjT.
    # Instead the host passes, per core: adjT_own = adj^T rows of the own
    # 512-node chunk (dense), and adjT = the full-global-order adj^T slice
    # with the own chunk rows ZEROED. The own-block matmul (which overlaps
    # the collective) uses adjT_own with the local state tile; the gathered
    # blocks run over all four chunks of adjT, where the own chunk
    # contributes zero. Static program, per-core data.
    adjT_own_d = nc.dram_tensor("adjT_own", [RPC, RPC], F32,
                                kind="ExternalInput").ap()

    with tile.TileContext(nc) as tc:
        with tc.tile_pool(name="persist", bufs=1) as pp, \
             tc.tile_pool(name="adjp", bufs=NKT + 4) as adjp, \
             tc.tile_pool(name="wp", bufs=2 * NL * 2) as wp, \
             tc.tile_pool(name="xnp", bufs=4) as xnp, \
             tc.tile_pool(name="xfp", bufs=8) as xfp, \
             tc.tile_pool(name="xtp", bufs=4) as xtp, \
             tc.tile_pool(name="msp", bufs=4) as msp, \
             tc.tile_pool(name="gthp", bufs=8) as gthp, \
             tc.tile_pool(name="smp", bufs=32) as smp, \
             tc.tile_pool(name="psM", bufs=2, space="PSUM") as psM, \
             tc.tile_pool(name="psG", bufs=4, space="PSUM") as psG, \
             tc.tile_pool(name="psT", bufs=2, space="PSUM") as psT, \
             tc.tile_pool(name="dr", bufs=4, space="DRAM") as dr:

            # ---------- constants ----------
            ident = pp.tile([128, 128], F32, name="ident")
            make_identity(nc, ident[:])
            ident_bf = pp.tile([128, 128], BF16, name="ident_bf")
            make_identity(nc, ident_bf[:])
            eps = pp.tile([1, 1], F32, name="eps")
            nc.vector.memset(eps[:], 1e-10)
            z128 = pp.tile([128, 1], F32, name="z128")
            nc.vector.memset(z128[:], 0.0)
            ones1 = pp.tile([1, 128], BF16, name="ones1")
            nc.vector.memset(ones1[:], 1.0)

            # ---------- bilinear interpolation -> x0 natural (own rows) ----
            # natural state layout: [128, 4*S]; row-tile t occupies columns
            # [t*S, (t+1)*S) = 256 state dims for the 128 points of tile t.
            x0n = xnp.tile([128, 4 * S], BF16, tag="xn", name="x0n")
            nc.vector.memset(x0n[:], 0.0)

            bp_sb = pp.tile([128, 8], F32, name="bp_sb")
            for t in range(4):
                nc.sync.dma_start(out=bp_sb[:, 2 * t:2 * t + 2],
                                  in_=bp_d[128 * t:128 * (t + 1), :])

            for t in range(4):
                bpx = bp_sb[:, 2 * t:2 * t + 1]
                bpy = bp_sb[:, 2 * t + 1:2 * t + 2]
                Xs = smp.tile([128, 1], F32, tag="sm")
                Ys = smp.tile([128, 1], F32, tag="sm")
                nc.vector.tensor_scalar_mul(out=Xs[:], in0=bpx, scalar1=float(W))
                nc.vector.tensor_scalar_mul(out=Ys[:], in0=bpy, scalar1=float(H))

                def floor_(v):
                    vi = smp.tile([128, 1], I32, tag="smi")
                    nc.vector.tensor_copy(out=vi[:], in_=v[:])
                    vf = smp.tile([128, 1], F32, tag="sm")
                    nc.vector.tensor_copy(out=vf[:], in_=vi[:])
                    corr = smp.tile([128, 1], F32, tag="sm")
                    nc.vector.tensor_tensor(out=corr[:], in0=vf[:], in1=v[:],
                                            op=OP.is_gt)
                    v0 = smp.tile([128, 1], F32, tag="sm")
                    nc.vector.tensor_tensor(out=v0[:], in0=vf[:], in1=corr[:],
                                            op=OP.subtract)
                    return v0

                X0f = floor_(Xs)
                Y0f = floor_(Ys)
                fx = smp.tile([128, 1], F32, tag="sm")
                fy = smp.tile([128, 1], F32, tag="sm")
                nc.vector.tensor_tensor(out=fx[:], in0=Xs[:], in1=X0f[:],
                                        op=OP.subtract)
                nc.vector.tensor_tensor(out=fy[:], in0=Ys[:], in1=Y0f[:],
                                        op=OP.subtract)
                gx = smp.tile([128, 1], F32, tag="sm")
                gy = smp.tile([128, 1], F32, tag="sm")
                nc.vector.tensor_scalar(out=gx[:], in0=fx[:], scalar1=-1.0,
                                        scalar2=1.0, op0=OP.mult, op1=OP.add)
                nc.vector.tensor_scalar(out=gy[:], in0=fy[:], scalar1=-1.0,
                                        scalar2=1.0, op0=OP.mult, op1=OP.add)
                ws = {}
                for nmx, wx in (("0", gx), ("1", fx)):
                    for nmy, wy in (("0", gy), ("1", fy)):
                        wt = smp.tile([128, 1], F32, tag="sm")
                        nc.vector.tensor_tensor(out=wt[:], in0=wx[:], in1=wy[:],
                                                op=OP.mult)
                        ws[nmx + nmy] = wt

                def clip_(v, hi):
                    vc = smp.tile([128, 1], F32, tag="sm")
                    nc.vector.tensor_scalar(out=vc[:], in0=v[:], scalar1=0.0,
                                            scalar2=float(hi), op0=OP.max,
                                            op1=OP.min)
                    return vc
                X1f = smp.tile([128, 1], F32, tag="sm")
                Y1f = smp.tile([128, 1], F32, tag="sm")
                nc.vector.tensor_scalar_add(out=X1f[:], in0=X0f[:], scalar1=1.0)
                nc.vector.tensor_scalar_add(out=Y1f[:], in0=Y0f[:], scalar1=1.0)
                Xc = {"0": clip_(X0f, W - 1), "1": clip_(X1f, W - 1)}
                Yc = {"0": clip_(Y0f, H - 1), "1": clip_(Y1f, H - 1)}
                acc = gthp.tile([128, C], F32, tag="acc")
                first = True
                for nmx in ("0", "1"):
                    for nmy in ("0", "1"):
                        idxf = smp.tile([128, 1], F32, tag="sm")
                        nc.vector.tensor_scalar(out=idxf[:], in0=Yc[nmy][:],
                                                scalar1=float(W), scalar2=None,
                                                op0=OP.mult)
                        nc.vector.tensor_tensor(out=idxf[:], in0=idxf[:],
                                                in1=Xc[nmx][:], op=OP.add)
                        idx = smp.tile([128, 1], I32, tag="smi")
                        nc.vector.tensor_copy(out=idx[:], in_=idxf[:])
                        g = gthp.tile([128, C], F32, tag="gth")
                        nc.gpsimd.indirect_dma_start(
                            out=g[:], out_offset=None, in_=feat,
                            in_offset=bass.IndirectOffsetOnAxis(ap=idx[:, :1],
                                                                axis=0))
                        if first:
                            nc.vector.tensor_scalar_mul(
                                out=acc[:], in0=g[:], scalar1=ws[nmx + nmy][:, :1])
                            first = False
                        else:
                            gw = gthp.tile([128, C], F32, tag="gw")
                            nc.vector.tensor_scalar_mul(
                                out=gw[:], in0=g[:], scalar1=ws[nmx + nmy][:, :1])
                            nc.vector.tensor_tensor(out=acc[:], in0=acc[:],
                                                    in1=gw[:], op=OP.add)
                nc.vector.tensor_copy(out=x0n[:, t * S:t * S + C], in_=acc[:])
                nc.vector.tensor_copy(out=x0n[:, t * S + C:t * S + C + 2],
                                      in_=bp_sb[:, 2 * t:2 * t + 2])

            # ---------- small parameter loads ----------
            bias_j0 = pp.tile([1, NL * S], F32, name="bias_j0")
            bias_j1 = pp.tile([1, NL * S], F32, name="bias_j1")
            nc.sync.dma_start(out=bias_j0[:], in_=bias_d[0:1, :])
            nc.sync.dma_start(out=bias_j1[:], in_=bias_d[1:2, :])
            bcomb = pp.tile([1, NL * S], F32, name="bcomb")
            nc.vector.tensor_tensor(out=bcomb[:], in0=bias_j0[:], in1=bias_j1[:],
                                    op=OP.add)
            bcomb_bf = pp.tile([1, NL * S], BF16, name="bcomb_bf")
            nc.vector.tensor_copy(out=bcomb_bf[:], in_=bcomb[:])
            wfc_sb = pp.tile([128, 4], BF16, name="wfc_sb")
            nc.gpsimd.dma_start(out=wfc_sb[:], in_=wfc_d)
            bfc_sb = pp.tile([2, 1], F32, name="bfc_sb")
            nc.sync.dma_start(out=bfc_sb[:], in_=bfc_d)
            maskT_sb = pp.tile([2, RPC], F32, name="maskT_sb")
            nc.sync.dma_start(out=maskT_sb[:], in_=maskT_d)
            bpT_sb = pp.tile([2, RPC], F32, name="bpT_sb")
            nc.sync.dma_start(out=bpT_sb[:], in_=bpT_d)

            def exchange(xn_tile):
                """AllGather the natural-layout own block; return (agout,
                x_full chunk tiles read back in global chunk order)."""
                agin = dr.tile([RPC, S], BF16, tag="agin")
                # per-row-tile writes so each can issue as soon as its
                # epilogue block lands, pipelining with the remaining fc2/ACTs
                for t_ in range(4):
                    nc.sync.dma_start(
                        out=agin[128 * t_:128 * (t_ + 1), :],
                        in_=xn_tile[:, t_ * S:(t_ + 1) * S])
                agout = dr.tile([N, S], BF16, tag="agout")
                nc.gpsimd.collective_compute(
                    "AllGather", OP.bypass, replica_groups=GROUPS,
                    ins=[agin[:].opt()], outs=[agout[:].opt()])
                xfull = []
                for r in range(4):
                    xr = xfp.tile([128, 4 * S], BF16, tag="xf",
                                  name=f"xr_{r}")
                    nc.scalar.dma_start(
                        out=xr[:].rearrange("p (t d) -> p t d", t=4),
                        in_=agout[RPC * r:RPC * (r + 1), :]
                        .rearrange("(t p) d -> p t d", p=128))
                    xfull.append(xr)
                return xfull

            xfull = exchange(x0n)

            # ---------- persistent adjacency + weights (overlap AG0) ------
            # adjT_sb[k]: global k-tile rows of adj^T own-column slice, with
            # the own chunk rows zeroed by the host.
            adjT_sb = [adjp.tile([128, RPC], BF16, tag="adjT", name=f"adjT{k}")
                       for k in range(NKT)]
            for k in range(NKT):
                nc.gpsimd.dma_start(out=adjT_sb[k][:],
                                    in_=adjT_d[128 * k:128 * (k + 1), :])
            # own-chunk rows (not zeroed), 4 k-tiles
            adjT_own = [adjp.tile([128, RPC], BF16, tag="adjT", name=f"adjTo{k}")
                        for k in range(4)]
            for k in range(4):
                nc.gpsimd.dma_start(out=adjT_own[k][:],
                                    in_=adjT_own_d[128 * k:128 * (k + 1), :])

            w_sb = {}
            for l in range(NL):
                for j in range(2):
                    for kt in range(2):
                        t_ = wp.tile([128, S], BF16, tag="w", name=f"w{l}_{j}_{kt}")
                        nc.gpsimd.dma_start(
                            out=t_[:],
                            in_=wst_d[l:l + 1, j:j + 1,
                                      128 * kt:128 * (kt + 1), :].opt())
                        w_sb[(l, j, kt)] = t_

            # ---------- gconv layers ----------
            xn_cur = x0n
            blockin = None
            for l in range(NL):
                if l == 0 or l == NL - 1:
                    kind = "plain"
                elif l % 2 == 1:
                    kind = "relu"
                    blockin = xn_cur
                else:
                    kind = "resid"

                # --- state transposes (local; overlap previous AG) ---
                xT = [xtp.tile([128, RPC], BF16, tag="xt", name=f"xT{l}_{i}")
                      for i in range(2)]
                for t in range(4):
                    for mt in range(2):
                        ptx = psT.tile([128, 128], BF16, tag="psT", space="PSUM")
                        nc.tensor.transpose(
                            out=ptx[:],
                            in_=xn_cur[:, t * S + mt * 128:t * S + (mt + 1) * 128],
                            identity=ident_bf[:])
                        nc.vector.tensor_copy(out=xT[mt][:, 128 * t:128 * (t + 1)],
                                              in_=ptx[:])

                # --- fc1 into psum (local; overlap previous AG) ---
                pg = [psG.tile([128, S], F32, tag="psG", space="PSUM",
                               name=f"pg{l}_{i}") for i in range(4)]
                for Mt in range(4):
                    for kt2 in range(2):
                        nc.tensor.matmul(
                            out=pg[Mt][:],
                            lhsT=xT[kt2][:, 128 * Mt:128 * (Mt + 1)],
                            rhs=w_sb[(l, 0, kt2)][:],
                            start=(kt2 == 0), stop=False)
                    # bias row (K=1 broadcast matmul)
                    nc.tensor.matmul(
                        out=pg[Mt][:], lhsT=ones1[:],
                        rhs=bcomb_bf[0:1, S * l:S * (l + 1)],
                        start=False, stop=False)

                # --- adjacency matmul m^T [2 x [128, 512]] ---
                pm = [psM.tile([128, RPC], F32, tag="psM", space="PSUM",
                               name=f"pm{l}_{i}") for i in range(2)]
                # own block first (local state, overlaps previous AG)
                for mt in range(2):
                    for kt in range(4):
                        nc.tensor.matmul(
                            out=pm[mt][:],
                            lhsT=xn_cur[:, kt * S + mt * 128:kt * S + (mt + 1) * 128],
                            rhs=adjT_own[kt][:],
                            start=(kt == 0), stop=False)
                # gathered blocks (own chunk rows of adjT_sb are zeroed)
                for r in range(4):
                    for kt in range(4):
                        gk = r * 4 + kt
                        for mt in range(2):
                            nc.tensor.matmul(
                                out=pm[mt][:],
                                lhsT=xfull[r][:, kt * S + mt * 128:
                                              kt * S + (mt + 1) * 128],
                                rhs=adjT_sb[gk][:],
                                start=False,
                                stop=(r == 3 and kt == 3))
                m_sb = []
                for mt in range(2):
                    ms = msp.tile([128, RPC], BF16, tag="ms")
                    nc.vector.tensor_copy(out=ms[:], in_=pm[mt][:])
                    m_sb.append(ms)

                # --- fc2 + epilogue ---
                xn_next = xnp.tile([128, 4 * S], BF16, tag="xn")
                for Mt in range(4):
                    for kt2 in range(2):
                        nc.tensor.matmul(
                            out=pg[Mt][:],
                            lhsT=m_sb[kt2][:, 128 * Mt:128 * (Mt + 1)],
                            rhs=w_sb[(l, 1, kt2)][:],
                            start=False, stop=(kt2 == 1))
                    dst = xn_next[:, Mt * S:(Mt + 1) * S]
                    if kind == "relu":
                        nc.scalar.activation(out=dst, in_=pg[Mt][:], func=AF.Relu,
                                             bias=z128[:])
                    elif kind == "plain":
                        nc.scalar.activation(out=dst, in_=pg[Mt][:],
                                             func=AF.Identity, bias=z128[:])
                    else:
                        t2 = msp.tile([128, S], BF16, tag="t2")
                        nc.scalar.activation(out=t2[:], in_=pg[Mt][:],
                                             func=AF.Identity, bias=z128[:])
                        t3 = msp.tile([128, S], BF16, tag="t2")
                        nc.vector.tensor_tensor(
                            out=t3[:], in0=t2[:],
                            in1=blockin[:, Mt * S:(Mt + 1) * S], op=OP.add)
                        nc.vector.tensor_scalar(out=dst, in0=t3[:], scalar1=0.0,
                                                scalar2=None, op0=OP.max)

                if l < NL - 1:
                    xfull = exchange(xn_next)
                xn_cur = xn_next

            # ---------- head: gcn_pred, pred_points ----------
            x14T = [xtp.tile([128, RPC], BF16, tag="xt", name=f"x14T_{i}")
                    for i in range(2)]
            for t in range(4):
                for mt in range(2):
                    ptx = psT.tile([128, 128], BF16, tag="psT", space="PSUM")
                    nc.tensor.transpose(
                        out=ptx[:],
                        in_=xn_cur[:, t * S + mt * 128:t * S + (mt + 1) * 128],
                        identity=ident_bf[:])
                    nc.vector.tensor_copy(out=x14T[mt][:, 128 * t:128 * (t + 1)],
                                          in_=ptx[:])
            pgc = psT.tile([2, RPC], F32, tag="psT", space="PSUM")
            nc.tensor.matmul(out=pgc[:], lhsT=wfc_sb[:, 0:2], rhs=x14T[0][:],
                             start=True, stop=False)
            nc.tensor.matmul(out=pgc[:], lhsT=wfc_sb[:, 2:4], rhs=x14T[1][:],
                             start=False, stop=True)
            gcn_T = pp.tile([2, RPC], F32, name="gcn_T")
            nc.scalar.activation(out=gcn_T[:], in_=pgc[:], func=AF.Identity,
                                 bias=bfc_sb[:])
            nc.sync.dma_start(out=out_gcn, in_=gcn_T[:])
            d_T = pp.tile([2, RPC], F32, name="d_T")
            nc.vector.tensor_tensor(out=d_T[:], in0=gcn_T[:], in1=maskT_sb[:],
                                    op=OP.mult)
            pred_T = pp.tile([2, RPC], F32, name="pred_T")
            nc.vector.tensor_tensor(out=pred_T[:], in0=bpT_sb[:], in1=d_T[:],
                                    op=OP.add)
            nc.sync.dma_start(out=out_pred, in_=pred_T[:])

            # ---------- laplacian energy ----------
            d_T_bf = pp.tile([2, RPC], BF16, name="d_T_bf")
            nc.vector.tensor_copy(out=d_T_bf[:], in_=d_T[:])
            agin_d = dr.tile([RPC, 2], BF16, tag="agind")
            for t in range(4):
                ptd = psT.tile([128, 2], BF16, tag="psT", space="PSUM")
                nc.tensor.transpose(out=ptd[:, 0:2],
                                    in_=d_T_bf[:, 128 * t:128 * (t + 1)],
                                    identity=ident_bf[0:2, 0:2])
                dnn = smp.tile([128, 2], BF16, tag="dn")
                nc.vector.tensor_copy(out=dnn[:], in_=ptd[:, 0:2])
                nc.sync.dma_start(out=agin_d[128 * t:128 * (t + 1), :], in_=dnn[:])
            agout_d = dr.tile([N, 2], BF16, tag="agoutd")
            nc.gpsimd.collective_compute(
                "AllGather", OP.bypass, replica_groups=GROUPS,
                ins=[agin_d[:].opt()], outs=[agout_d[:].opt()])
            # (adj @ d)^T via global-order adjT (own rows zeroed) + own rows
            pad = psT.tile([2, RPC], F32, tag="psT", space="PSUM")
            for kt in range(NKT):
                dn = smp.tile([128, 2], BF16, tag="dn")
                nc.sync.dma_start(out=dn[:],
                                  in_=agout_d[128 * kt:128 * (kt + 1), :])
                nc.tensor.matmul(out=pad[:], lhsT=dn[:], rhs=adjT_sb[kt][:],
                                 start=(kt == 0), stop=False)
            # own-chunk contribution (host zeroed those rows in adjT_d):
            # own rows of d in global order are unknown per-core, BUT
            # adjT_own covers exactly the own chunk rows; the own-chunk d
            # values are d_nat own = transposed d_T (already in agin_d
            # order = own rows 0..511). Read them back from agin_d.
            for kt in range(4):
                dno = smp.tile([128, 2], BF16, tag="dn")
                nc.sync.dma_start(out=dno[:],
                                  in_=agin_d[128 * kt:128 * (kt + 1), :])
                nc.tensor.matmul(out=pad[:], lhsT=dno[:], rhs=adjT_own[kt][:],
                                 start=False, stop=(kt == 3))
            lapd = pp.tile([2, RPC], F32, name="lapd")
            nc.vector.tensor_tensor(out=lapd[:], in0=d_T[:], in1=pad[:],
                                    op=OP.subtract)
            sq = pp.tile([2, RPC], F32, name="sq")
            nc.vector.tensor_tensor(out=sq[:], in0=lapd[:], in1=lapd[:],
                                    op=OP.mult)
            ones2 = pp.tile([2, 1], F32, name="ones2")
            nc.vector.memset(ones2[:], 1.0)
            pe2 = psT.tile([1, RPC], F32, tag="psT", space="PSUM")
            nc.tensor.matmul(out=pe2[:], lhsT=ones2[:], rhs=sq[:],
                             start=True, stop=True)
            esb = pp.tile([1, RPC], F32, name="esb")
            part = pp.tile([1, 1], F32, name="part")
            nc.scalar.activation(out=esb[:], in_=pe2[:], func=AF.Sqrt,
                                 bias=eps[:], accum_out=part[:])
            agin_e_sb = pp.tile([1, 16], F32, name="agin_e_sb")
            nc.vector.memset(agin_e_sb[:], 0.0)
            nc.vector.tensor_scalar_mul(out=agin_e_sb[0:1, 0:1], in0=part[:],
                                        scalar1=1.0 / N)
            agin_e = dr.tile([1, 16], F32, tag="agine")
            nc.sync.dma_start(out=agin_e[:], in_=agin_e_sb[:])
            agout_e = dr.tile([4, 16], F32, tag="agoute")
            nc.gpsimd.collective_compute(
                "AllGather", OP.bypass, replica_groups=GROUPS,
                ins=[agin_e[:].opt()], outs=[agout_e[:].opt()])
            pe4 = pp.tile([4, 16], F32, name="pe4")
            nc.sync.dma_start(out=pe4[:], in_=agout_e[:])
            ones4 = pp.tile([4, 1], F32, name="ones4")
            nc.vector.memset(ones4[:], 1.0)
            plap = psT.tile([1, 1], F32, tag="psT", space="PSUM")
            nc.tensor.matmul(out=plap[:], lhsT=pe4[:, 0:1], rhs=ones4[:],
                             start=True, stop=True)
            lap_sb = pp.tile([1, 1], F32, name="lap_sb")
            nc.vector.tensor_copy(out=lap_sb[:], in_=plap[:])
            nc.sync.dma_start(out=out_lap, in_=lap_sb[:])

    nc.compile()
    nc.m = get_hw_module(nc.m)
    _BUILD_CACHE["nc"] = nc
    return nc


def _prep_in_maps(inputs):
    features = np.asarray(inputs["features"], np.float32)
    base_point = np.asarray(inputs["base_point"], np.float32)
    adj = np.asarray(inputs["adj"], np.float32)
    mask = np.asarray(inputs["point_mask"], np.float32)
    w_first = np.asarray(inputs["w_first"], np.float32)
    b_first = np.asarray(inputs["b_first"], np.float32)
    w_mid = np.asarray(inputs["w_mid"], np.float32)
    b_mid = np.asarray(inputs["b_mid"], np.float32)
    w_last = np.asarray(inputs["w_last"], np.float32)
    b_last = np.asarray(inputs["b_last"], np.float32)
    w_fc = np.asarray(inputs["w_fc"], np.float32)
    b_fc = np.asarray(inputs["b_fc"], np.float32)

    wstack = np.zeros((NL, 2, S, S), np.float32)
    bstack = np.zeros((NL, 2, S), np.float32)
    wstack[0, :, :w_first.shape[1], :] = w_first
    bstack[0] = b_first
    for i in range(MID):
        wstack[1 + 2 * i, 0] = w_mid[i, 0]; wstack[1 + 2 * i, 1] = w_mid[i, 1]
        wstack[2 + 2 * i, 0] = w_mid[i, 2]; wstack[2 + 2 * i, 1] = w_mid[i, 3]
        bstack[1 + 2 * i, 0] = b_mid[i, 0]; bstack[1 + 2 * i, 1] = b_mid[i, 1]
        bstack[2 + 2 * i, 0] = b_mid[i, 2]; bstack[2 + 2 * i, 1] = b_mid[i, 3]
    wstack[NL - 1] = w_last
    bstack[NL - 1] = b_last
    bias_host = np.zeros((2, NL * S), np.float32)
    for l in range(NL):
        for j in range(2):
            bias_host[j, l * S:(l + 1) * S] = bstack[l, j]
    wfc_host = np.concatenate([w_fc[0:128, :], w_fc[128:256, :]], axis=1)
    bfc_host = b_fc[:, None]

    feats = [np.ascontiguousarray(features[b].reshape(C, H * W).T)
             for b in range(B)]
    in_maps = []
    for c in range(8):
        b, r = c // 4, c % 4
        rows = slice(r * RPC, (r + 1) * RPC)
        adjT_full = np.ascontiguousarray(adj[b, rows, :].T)   # [2048, 512]
        adjT_own = np.ascontiguousarray(adjT_full[rows, :])   # [512, 512]
        adjT_masked = adjT_full.copy()
        adjT_masked[rows, :] = 0.0
        in_maps.append({
            "feat": feats[b],
            "adjT": adjT_masked,
            "adjT_own": adjT_own,
            "bp": np.ascontiguousarray(base_point[b, rows, :]),
            "bpT": np.ascontiguousarray(base_point[b, rows, :].T),
            "maskT": np.ascontiguousarray(
                np.broadcast_to(mask[b, 0, rows, 0][None, :], (2, RPC))),
            "wst": wstack,
            "bias": bias_host,
            "wfc": wfc_host,
            "bfc": bfc_host,
        })
    return in_maps


def _install_profile_hook():
    """Register the axon NTFF profile hook (missing antenv.axon_hooks in
    this image) so run_bass_kernel_spmd(trace=True) can report exec_time_ns."""
    import sys
    import types
    import contextlib
    import ctypes

    if "antenv.axon_hooks" in sys.modules:
        return
    so_path = "/opt/axon/libaxon_pjrt.so"
    lib = ctypes.CDLL(so_path)
    if not hasattr(lib, "axon_start_nrt_profile"):
        return
    lib.axon_start_nrt_profile.argtypes = [ctypes.POINTER(ctypes.c_int64),
                                           ctypes.c_size_t]
    lib.axon_start_nrt_profile.restype = ctypes.c_int64
    lib.axon_stop_nrt_profile.argtypes = [ctypes.c_char_p]
    lib.axon_stop_nrt_profile.restype = ctypes.c_int64

    @contextlib.contextmanager
    def _hook(output_dir, device_ids):
        import jax
        jax.devices()
        if device_ids:
            ids = (ctypes.c_int64 * len(device_ids))(*device_ids)
            rc = lib.axon_start_nrt_profile(ids, len(device_ids))
        else:
            rc = lib.axon_start_nrt_profile(None, 0)
        if rc != 0:
            raise RuntimeError(f"axon_start_nrt_profile rc={rc}")
        try:
            yield
        finally:
            n = lib.axon_stop_nrt_profile(str(output_dir).encode())
            print(f"profile: {n} file(s) written to {output_dir}")

    mod = types.ModuleType("antenv.axon_hooks")
    mod.get_axon_ntff_profile_hook = lambda: _hook
    mod.set_axon_ntff_profile_hook = lambda h: None
    sys.modules["antenv.axon_hooks"] = mod

    from concourse import bass_utils as bu
    bu.upload_artifacts = lambda tmpdir: str(tmpdir)


def kernel(**inputs):
    import os
    nc = _build()
    in_maps = _prep_in_maps(inputs)
    trace = bool(os.environ.get("KERNEL_TRACE"))
    if trace:
        _install_profile_hook()
    res = run_bass_kernel_spmd(nc, in_maps, core_ids=list(range(8)),
                               trace=trace)
    if trace:
        print(f"HW exec time: {res.exec_time_ns} ns "
              f"(mean {res.mean_exec_time_ns}, max core "
              f"{res.max_exec_time_core_id})")
    pred = np.zeros((B, N, 2), np.float32)
    gcn = np.zeros((B, N, 2), np.float32)
    lap = np.zeros((B,), np.float32)
    for c in range(8):
        b, r = c // 4, c % 4
        rows = slice(r * RPC, (r + 1) * RPC)
        pred[b, rows, :] = res.results[c]["out_pred"].T
        gcn[b, rows, :] = res.results[c]["out_gcn"].T
        if r == 0:
            lap[b] = res.results[c]["out_lap"][0, 0]
    return pred, gcn, lap
